# revision 2
# baseline (speedup 1.0000x reference)
"""Trainium2 Bass kernel for the Kruskal (CP/Tucker) linear layer.

Math: the reference reconstructs W (4096x4096) from a rank-16 CP core and
Tucker factors, then computes y = x @ W.T + bias.  Because the 6D core is a
CP (Kruskal) tensor of rank 16, W itself is exactly rank 16:

    W = g_out @ g_in.T
    g_in[def, r]  = (f3@c3)[d,r] * (f4@c4)[e,r] * (f5@c5)[f,r]   (4096 x 16)
    g_out[abc, r] = (f0@c0)[a,r] * (f1@c1)[b,r] * (f2@c2)[c,r]   (4096 x 16)

so  y = (x @ g_in) @ g_out.T + bias.  The device kernel computes the two
x-dependent projections; the tiny factor-only products (g_in/g_out, ~100
KFLOP) are prepared on the host.

Sharding: data-parallel over the batch (4096 rows -> 8 cores x 512). No
collectives.  Per core:
  1. SWDGE cast-DMA x tile (128,4096) fp32 -> SBUF bf16
  2. xbar DMA-transpose (SBUF->SBUF) -> x^T tiles (features on partitions)
  3. stage 1: 32 accumulating matmuls  t^T(16,512) += g_in_kt.T @ x^T_kt
  4. stage 2: K=17 matmuls (rank 16 + bias row)  y = [t,1] @ [g_out.T; bias]
  5. DVE copy PSUM->SBUF, DMA y fp32 out
"""

import numpy as np
import ml_dtypes

N_CORES = 8
BATCH = 4096
D = 4096          # in/out features (16*16*16)
R = 16            # CP rank
P = 128           # partitions
NB = BATCH // N_CORES   # 512 batch rows per core
BT = NB // P            # 4 batch tiles per core
KT = D // P             # 32 feature k-tiles
NT = 512                # output column tile (fp32 moving-operand max)
JT = D // NT            # 8 output column tiles

_PROGRAM = None


def _build_program():
    import concourse.tile as tile
    from concourse import bacc, mybir

    nc = bacc.Bacc(
        "TRN2",
        target_bir_lowering=False,
        debug=False,
        enable_asserts=False,
        num_devices=N_CORES,
    )
    x_d = nc.dram_tensor("xc", (NB, D), mybir.dt.float32, kind="ExternalInput")
    gin_d = nc.dram_tensor("gin", (P, KT * R), mybir.dt.bfloat16, kind="ExternalInput")
    gout_d = nc.dram_tensor("goutT", (R + 1, D), mybir.dt.float32, kind="ExternalInput")
    y_d = nc.dram_tensor("yc", (NB, D), mybir.dt.float32, kind="ExternalOutput")

    with tile.TileContext(nc) as tc:
        with (
            tc.tile_pool(name="const", bufs=1) as constp,
            tc.tile_pool(name="xb", bufs=2) as xbp,
            tc.tile_pool(name="xT", bufs=2) as xTp,
            tc.tile_pool(name="ysb", bufs=2) as ysbp,
            tc.tile_pool(name="tpsum", bufs=1, space="PSUM") as tpsump,
            tc.tile_pool(name="ypsum", bufs=2, space="PSUM") as ypsump,
        ):
            gin_sb = constp.tile([P, KT * R], mybir.dt.bfloat16)
            nc.sync.dma_start(gin_sb[:], gin_d.ap())
            gout_sb = constp.tile([R + 1, D], mybir.dt.float32)
            nc.sync.dma_start(gout_sb[:], gout_d.ap())

            # t^T staging: rows 0..15 = (x@g_in).T, row 16 = ones (bias row).
            # memset the whole tile (DVE needs start partition 0); rows 0..15
            # are overwritten by the PSUM copy below.
            tT_sb = constp.tile([R + 1, NB], mybir.dt.float32)
            nc.vector.memset(tT_sb[:], 1.0)

            tT_ps = tpsump.tile([R, NB], mybir.dt.float32)
            for bt in range(BT):
                xb = xbp.tile([P, D], mybir.dt.bfloat16)
                # SWDGE cast fp32 -> bf16 while loading
                nc.gpsimd.dma_start(xb[:], x_d.ap()[bt * P : (bt + 1) * P, :])
                xT = xTp.tile([P, KT, P], mybir.dt.bfloat16)
                # xbar transpose: xT[p, kt, b] = xb[b, kt*128 + p]
                nc.sync.dma_start(xT[:], xb[:], transpose=True)
                for kt in range(KT):
                    nc.tensor.matmul(
                        tT_ps[:, bt * P : (bt + 1) * P],
                        lhsT=gin_sb[:, kt * R : (kt + 1) * R],
                        rhs=xT[:, kt, :],
                        start=(kt == 0),
                        stop=(kt == KT - 1),
                    )
            nc.vector.tensor_copy(tT_sb[0:R, :], tT_ps[:])

            for bt in range(BT):
                y_sb = ysbp.tile([P, D], mybir.dt.float32)
                for jt in range(JT):
                    y_ps = ypsump.tile([P, NT], mybir.dt.float32)
                    nc.tensor.matmul(
                        y_ps[:],
                        lhsT=tT_sb[:, bt * P : (bt + 1) * P],
                        rhs=gout_sb[:, jt * NT : (jt + 1) * NT],
                    )
                    nc.vector.tensor_copy(y_sb[:, jt * NT : (jt + 1) * NT], y_ps[:])
                nc.sync.dma_start(y_d.ap()[bt * P : (bt + 1) * P, :], y_sb[:])

    nc.compile()
    return nc


def _get_program():
    global _PROGRAM
    if _PROGRAM is None:
        _PROGRAM = _build_program()
    return _PROGRAM


def _host_factors(inputs):
    """Build g_in (SBUF layout, bf16) and [g_out.T; bias] (fp32) on host."""
    c = [np.asarray(inputs[f"c{i}"], dtype=np.float64) for i in range(6)]
    f = [np.asarray(inputs[f"f{i}"], dtype=np.float64) for i in range(6)]
    bias = np.asarray(inputs["bias"], dtype=np.float32)
    h = [f[i] @ c[i] for i in range(6)]  # (16,16) each
    g_out = (
        h[0][:, None, None, :] * h[1][None, :, None, :] * h[2][None, None, :, :]
    ).reshape(D, R)
    g_in = (
        h[3][:, None, None, :] * h[4][None, :, None, :] * h[5][None, None, :, :]
    ).reshape(D, R)
    # gin SBUF layout: gin_l[p, kt*R + r] = g_in[kt*128 + p, r]
    gin_l = np.ascontiguousarray(
        g_in.reshape(KT, P, R).transpose(1, 0, 2).reshape(P, KT * R)
    ).astype(ml_dtypes.bfloat16)
    goutT = np.concatenate(
        [g_out.T.astype(np.float32), bias[None, :]], axis=0
    ).astype(np.float32)  # (17, 4096)
    return gin_l, goutT


# test-harness hooks (unused in graded path)
TRACE = False
LAST_RESULTS = None


def kernel(**inputs):
    from concourse.bass_utils import run_bass_kernel_spmd

    global LAST_RESULTS
    x = np.ascontiguousarray(np.asarray(inputs["x"], dtype=np.float32))
    gin_l, goutT = _host_factors(inputs)
    nc = _get_program()
    in_maps = [
        {
            "xc": np.ascontiguousarray(x[ci * NB : (ci + 1) * NB]),
            "gin": gin_l,
            "goutT": goutT,
        }
        for ci in range(N_CORES)
    ]
    res = run_bass_kernel_spmd(
        nc, in_maps, core_ids=list(range(N_CORES)), trace=TRACE
    )
    LAST_RESULTS = res
    y = np.concatenate([r["yc"] for r in res.results], axis=0)
    return np.ascontiguousarray(y.astype(np.float32))


if __name__ == "__main__":
    # quick smoke test with random data
    rng = np.random.default_rng(0)
    ins = {"x": rng.normal(size=(BATCH, D)).astype(np.float32)}
    for i in range(6):
        ins[f"c{i}"] = (rng.normal(size=(8, 16)) * 0.1).astype(np.float32)
        ins[f"f{i}"] = (rng.normal(size=(16, 8)) * 0.1).astype(np.float32)
    ins["bias"] = np.zeros(D, dtype=np.float32)
    y = kernel(**ins)
    print("y", y.shape, y.dtype)


# revision 8
# speedup vs baseline: 522.8906x; 522.8906x over previous
"""Trainium2 Bass kernel for the Kruskal (CP/Tucker) linear layer.

Math: the reference reconstructs W (4096x4096) from a rank-16 CP core and
Tucker factors, then computes y = x @ W.T + bias.  Because the 6D core is a
CP (Kruskal) tensor of rank 16, W itself is exactly rank 16:

    W = g_out @ g_in.T
    g_in[def, r]  = (f3@c3)[d,r] * (f4@c4)[e,r] * (f5@c5)[f,r]   (4096 x 16)
    g_out[abc, r] = (f0@c0)[a,r] * (f1@c1)[b,r] * (f2@c2)[c,r]   (4096 x 16)

so  y = (x @ g_in) @ g_out.T + bias.  The device kernel computes the two
x-dependent projections; the tiny factor-only products (g_in/g_out, ~100
KFLOP) are prepared on the host.

Sharding: data-parallel over the batch (4096 rows -> 8 cores x 512). No
collectives.  Per core:
  1. SWDGE cast-DMA x tile (128,4096) fp32 -> SBUF bf16
  2. xbar DMA-transpose (SBUF->SBUF) -> x^T tiles (features on partitions)
  3. stage 1: 32 accumulating matmuls  t^T(16,512) += g_in_kt.T @ x^T_kt
  4. stage 2: K=17 matmuls (rank 16 + bias row)  y = [t,1] @ [g_out.T; bias]
  5. DVE copy PSUM->SBUF, DMA y fp32 out
"""

import numpy as np
import ml_dtypes

N_CORES = 8
BATCH = 4096
D = 4096          # in/out features (16*16*16)
R = 16            # CP rank
P = 128           # partitions
NB = BATCH // N_CORES   # 512 batch rows per core
BT = NB // P            # 4 batch tiles per core
KT = D // P             # 32 feature k-tiles
NT = 512                # output column tile (fp32 moving-operand max)
JT = D // NT            # 8 output column tiles

_PROGRAM = None


def _build_program():
    import concourse.tile as tile
    from concourse import bacc, mybir

    nc = bacc.Bacc(
        "TRN2",
        target_bir_lowering=False,
        debug=False,
        enable_asserts=False,
        num_devices=N_CORES,
    )
    x_d = nc.dram_tensor("xc", (NB, D), mybir.dt.float32, kind="ExternalInput")
    gin_d = nc.dram_tensor("gin", (P, KT * R), mybir.dt.bfloat16, kind="ExternalInput")
    gout_d = nc.dram_tensor("goutT", (R + 1, D), mybir.dt.float32r, kind="ExternalInput")
    # aux row: [e16 (17 cols: zeros, col16=1), ones (128 cols)] used to write
    # the bias ones-row of t^T via a K=1 matmul (walrus rejects fp32r memset)
    aux_d = nc.dram_tensor("aux", (1, R + 1 + P), mybir.dt.bfloat16, kind="ExternalInput")
    y_d = nc.dram_tensor("yc", (NB, D), mybir.dt.float32, kind="ExternalOutput")

    with tile.TileContext(nc) as tc:
        with (
            tc.tile_pool(name="const", bufs=1) as constp,
            tc.tile_pool(name="xb", bufs=3) as xbp,
            tc.tile_pool(name="xT", bufs=3) as xTp,
            tc.tile_pool(name="tsb", bufs=2) as tsbp,
            tc.tile_pool(name="ysb", bufs=3) as ysbp,
            tc.tile_pool(name="tpsum", bufs=2, space="PSUM") as tpsump,
            tc.tile_pool(name="ypsum", bufs=2, space="PSUM") as ypsump,
        ):
            gin_sb = constp.tile([P, KT * R], mybir.dt.bfloat16)
            nc.sync.dma_start(gin_sb[:], gin_d.ap())
            gout_sb = constp.tile([R + 1, D], mybir.dt.float32r)
            nc.sync.dma_start(gout_sb[:], gout_d.ap())
            aux_sb = constp.tile([1, R + 1 + P], mybir.dt.bfloat16)
            nc.sync.dma_start(aux_sb[:], aux_d.ap())

            # fully pipelined per batch-tile: cast -> transpose -> stage1 ->
            # t copy -> stage2 -> y copies -> y store
            for bt in range(BT):
                xb = xbp.tile([P, D], mybir.dt.bfloat16)
                # SWDGE cast fp32 -> bf16 while loading
                nc.gpsimd.dma_start(xb[:], x_d.ap()[bt * P : (bt + 1) * P, :])
                xT = xTp.tile([P, KT, P], mybir.dt.bfloat16)
                # xbar transpose: xT[p, kt, b] = xb[b, kt*128 + p]
                nc.sync.dma_start(xT[:], xb[:], transpose=True)

                tT_ps = tpsump.tile([R + 1, P], mybir.dt.float32)
                # K=1 matmul writes ones into row 16 and zeros rows 0..15
                # (start=True), which the stage-1 matmuls then accumulate into
                nc.tensor.matmul(
                    tT_ps[:],
                    lhsT=aux_sb[:, 0 : R + 1],
                    rhs=aux_sb[:, R + 1 : R + 1 + P],
                    start=True,
                    stop=False,
                    skip_group_check=True,
                )
                for kt in range(KT):
                    nc.tensor.matmul(
                        tT_ps[0:R, :],
                        lhsT=gin_sb[:, kt * R : (kt + 1) * R],
                        rhs=xT[:, kt, :],
                        start=False,
                        stop=(kt == KT - 1),
                        skip_group_check=True,
                    )
                # t^T rows 0..15 = (x@g_in).T slice, row 16 = ones (bias row)
                tT_sb = tsbp.tile([R + 1, P], mybir.dt.float32r)
                nc.vector.tensor_copy(tT_sb[:], tT_ps[:])

                y_sb = ysbp.tile([P, D], mybir.dt.float32)
                for jt in range(JT):
                    y_ps = ypsump.tile([P, NT], mybir.dt.float32)
                    nc.tensor.matmul(
                        y_ps[:],
                        lhsT=tT_sb[:],
                        rhs=gout_sb[:, jt * NT : (jt + 1) * NT],
                    )
                    # split PSUM->SBUF copies across DVE and ACT engines
                    if jt % 2 == 0:
                        nc.vector.tensor_copy(
                            y_sb[:, jt * NT : (jt + 1) * NT], y_ps[:]
                        )
                    else:
                        nc.scalar.copy(y_sb[:, jt * NT : (jt + 1) * NT], y_ps[:])
                nc.sync.dma_start(y_d.ap()[bt * P : (bt + 1) * P, :], y_sb[:])

    nc.compile()
    return nc


def _get_program():
    global _PROGRAM
    if _PROGRAM is None:
        _PROGRAM = _build_program()
    return _PROGRAM


def _host_factors(inputs):
    """Build g_in (SBUF layout, bf16) and [g_out.T; bias] (fp32) on host."""
    c = [np.asarray(inputs[f"c{i}"], dtype=np.float64) for i in range(6)]
    f = [np.asarray(inputs[f"f{i}"], dtype=np.float64) for i in range(6)]
    bias = np.asarray(inputs["bias"], dtype=np.float32)
    h = [f[i] @ c[i] for i in range(6)]  # (16,16) each
    g_out = (
        h[0][:, None, None, :] * h[1][None, :, None, :] * h[2][None, None, :, :]
    ).reshape(D, R)
    g_in = (
        h[3][:, None, None, :] * h[4][None, :, None, :] * h[5][None, None, :, :]
    ).reshape(D, R)
    # gin SBUF layout: gin_l[p, kt*R + r] = g_in[kt*128 + p, r]
    gin_l = np.ascontiguousarray(
        g_in.reshape(KT, P, R).transpose(1, 0, 2).reshape(P, KT * R)
    ).astype(ml_dtypes.bfloat16)
    goutT = np.concatenate(
        [g_out.T.astype(np.float32), bias[None, :]], axis=0
    ).astype(np.float32)  # (17, 4096)
    aux = np.zeros((1, R + 1 + P), dtype=ml_dtypes.bfloat16)
    aux[0, R] = 1.0
    aux[0, R + 1 :] = 1.0
    return gin_l, goutT, aux


# test-harness hooks (unused in graded path)
TRACE = False
LAST_RESULTS = None


def kernel(**inputs):
    from concourse.bass_utils import run_bass_kernel_spmd

    global LAST_RESULTS
    x = np.ascontiguousarray(np.asarray(inputs["x"], dtype=np.float32))
    gin_l, goutT, aux = _host_factors(inputs)
    nc = _get_program()
    in_maps = [
        {
            "xc": np.ascontiguousarray(x[ci * NB : (ci + 1) * NB]),
            "gin": gin_l,
            "goutT": goutT,
            "aux": aux,
        }
        for ci in range(N_CORES)
    ]
    res = run_bass_kernel_spmd(
        nc, in_maps, core_ids=list(range(N_CORES)), trace=TRACE
    )
    LAST_RESULTS = res
    y = np.concatenate([r["yc"] for r in res.results], axis=0)
    return np.ascontiguousarray(y.astype(np.float32))


if __name__ == "__main__":
    # quick smoke test with random data
    rng = np.random.default_rng(0)
    ins = {"x": rng.normal(size=(BATCH, D)).astype(np.float32)}
    for i in range(6):
        ins[f"c{i}"] = (rng.normal(size=(8, 16)) * 0.1).astype(np.float32)
        ins[f"f{i}"] = (rng.normal(size=(16, 8)) * 0.1).astype(np.float32)
    ins["bias"] = np.zeros(D, dtype=np.float32)
    y = kernel(**ins)
    print("y", y.shape, y.dtype)
